# revision 6
# baseline (speedup 1.0000x reference)
"""Trainium2 Bass kernel for BERT factorized attention (v2, fp16).

Reference math (per batch b, head h, S=4096, H=1024, NH=16, HD=64):
    q = x @ Wq + bq ; k = x @ Wk + bk ; v = x @ Wv + bv
    s_probs = softmax_S(qT_head)            # [HD, S]
    c_probs = softmax_HD(k_head)            # [S, HD]
    s_ctx   = s_probs @ v_head              # [HD, HD]
    out     = c_probs @ s_ctx               # [S, HD]

Kernel strategy (one batch element per NeuronCore, 8 cores, no collectives):
  - x chunk -> fp16 convert -> PE-transpose (fp16, 1 cyc/row) -> xt.
  - All matmuls in fp16 (1 cyc/row at any width): QV projections with
    xt stationary, K projection with Wk stationary producing EKT=[h,s]
    directly; exp activations write fp16.
  - EKT lives entirely in SBUF (64KB/partition) — no DRAM scratch.
  - s-softmax denominators via ones-augmented V (phase2 psum cols 64/129).
  - c-softmax denominators folded into pass B: sctx is augmented with a
    block-diagonal ones column pair, so each pass-B matmul emits
    [ctx_unnorm | den_headA | den_headB]; one reciprocal + one broadcast
    multiply per chunk normalizes.
  - exp without max-subtraction is safe: q,k ~ N(0,1), fp16 max 65504.
"""

import sys

sys.path.insert(0, "/opt/trn_rl_repo")

import contextlib
from contextlib import ExitStack

import numpy as np

import concourse.bass as bass
import concourse.mybir as mybir
import concourse.tile as tile
from concourse import bacc, bass_utils
from concourse.masks import make_identity

F32 = mybir.dt.float32
FP16 = mybir.dt.float16

B, S, H = 8, 4096, 1024
NH, HD = 16, 64
STRIPE = 512
CPS = STRIPE // 128  # chunks per stripe
KT = H // 128  # contraction tiles
NP = NH // 2  # head pairs

EXPF = mybir.ActivationFunctionType.Exp
COPYF = mybir.ActivationFunctionType.Copy


def _bcast(ap_2d, n):
    """[p, c] AP -> [p, c, n] with step-0 broadcast on the last dim."""
    return bass.AP(
        tensor=ap_2d.tensor,
        offset=ap_2d.offset,
        ap=[ap_2d.ap[0], ap_2d.ap[1], [0, n]],
    )


def build_kernel(seq_len=S, with_bias=False, loop_n=None):
    """Build + compile the single-core program (SPMD across 8 cores)."""
    s = seq_len
    n_stripes = s // STRIPE
    n_chunks = s // 128

    nc = bacc.Bacc("TRN2", target_bir_lowering=False, debug=False, num_devices=8)

    x_d = nc.dram_tensor("x", [s, H], F32, kind="ExternalInput").ap()
    m_d = nc.dram_tensor("mask", [s], F32, kind="ExternalInput").ap()
    wq_d = nc.dram_tensor("wq", [H, H], F32, kind="ExternalInput").ap()
    wk_d = nc.dram_tensor("wk", [H, H], F32, kind="ExternalInput").ap()
    wv_d = nc.dram_tensor("wv", [H, H], F32, kind="ExternalInput").ap()
    if with_bias:
        bq_d = nc.dram_tensor("bq", [H], F32, kind="ExternalInput").ap()
        bk_d = nc.dram_tensor("bk", [H], F32, kind="ExternalInput").ap()
        bv_d = nc.dram_tensor("bv", [H], F32, kind="ExternalInput").ap()
    out_d = nc.dram_tensor("out", [s, H], F32, kind="ExternalOutput").ap()

    with tile.TileContext(nc) as tc:
        with ExitStack() as ctx:
            singles = ctx.enter_context(tc.tile_pool(name="singles", bufs=1))
            xpool = ctx.enter_context(tc.tile_pool(name="xpool", bufs=3))
            xhpool = ctx.enter_context(tc.tile_pool(name="xhpool", bufs=2))
            xtpool = ctx.enter_context(tc.tile_pool(name="xtpool", bufs=2))
            eqpool = ctx.enter_context(tc.tile_pool(name="eqpool", bufs=6))
            vapool = ctx.enter_context(tc.tile_pool(name="vapool", bufs=6))
            opool = ctx.enter_context(tc.tile_pool(name="opool", bufs=3))
            small = ctx.enter_context(tc.tile_pool(name="small", bufs=4))
            # PSUM (8 banks): tp 2 (transposes + phase2) + proj 4 + ktp 2
            tp = ctx.enter_context(tc.tile_pool(name="tp", bufs=2, space="PSUM"))
            proj = ctx.enter_context(tc.tile_pool(name="proj", bufs=4, space="PSUM"))
            ktp = ctx.enter_context(tc.tile_pool(name="ktp", bufs=2, space="PSUM"))
            p2p = tp

            identh = singles.tile([128, 128], FP16)
            make_identity(nc, identh)

            mask_sb = singles.tile([128, n_chunks], F32)
            nc.gpsimd.dma_start(out=mask_sb, in_=m_d.rearrange("(c p) -> p c", p=128))

            # weights: DMA fp32 staging chunks, convert to fp16
            w_r = {}
            for name, wd in (("wq", wq_d), ("wv", wv_d), ("wk", wk_d)):
                wr = singles.tile([128, KT, H], FP16, tag=f"{name}_r")
                w_r[name] = wr
                for k in range(KT):
                    st = opool.tile([128, H], F32, tag="ob")
                    nc.gpsimd.dma_start(out=st, in_=wd[k * 128 : (k + 1) * 128, :])
                    nc.scalar.activation(wr[:, k, :], st, COPYF)
            wq_r, wk_r, wv_r = w_r["wq"], w_r["wk"], w_r["wv"]

            if with_bias:
                bqb = singles.tile([128, H], F32)
                bvb = singles.tile([128, H], F32)
                for bt, bd in ((bqb, bq_d), (bvb, bv_d)):
                    src = bass.AP(
                        tensor=bd.tensor, offset=bd.offset, ap=[[0, 128], bd.ap[0]]
                    )
                    nc.sync.dma_start(out=bt, in_=src)
                bkc = singles.tile([128, KT], F32)
                nc.sync.dma_start(out=bkc, in_=bk_d.rearrange("(t p) -> p t", p=128))

            # EKT resident in SBUF: [d-pair partition, head-pair, s]
            ekt_sb = singles.tile([128, KT, s], FP16)
            acc = singles.tile([128, NP, 130], F32)
            # sctx: [128, NP, 130] fp16; cols 0:128 = block-diag s_ctx,
            # cols 128:130 = block-diag ones (denominator probe for pass B)
            sctx = singles.tile([128, NP, 130], FP16)
            ones16 = singles.tile([128, 16, 1], FP16)
            zcol = singles.tile([128, 1], F32)
            nc.vector.memset(zcol, 0.0)
            onecol = singles.tile([128, 1], F32)
            nc.vector.memset(onecol, 1.0)

            def _rep(col, *dims):
                """[p,1] f32 tile -> step-0 broadcast AP over extra dims."""
                return bass.AP(
                    tensor=col.tensor,
                    offset=col.offset,
                    ap=[col.ap[0]] + [[0, d] for d in dims],
                )

            nc.vector.tensor_copy(ones16, _rep(onecol, 16, 1))
            # zero the full sctx tile once; ones cols written once (persist)
            nc.vector.tensor_copy(
                sctx[:].rearrange("p a b -> p (a b)"), _rep(zcol, NP * 130)
            )
            nc.vector.tensor_copy(sctx[0:64, :, 128:129], ones16[0:64, 0:NP, :])
            nc.vector.tensor_copy(sctx[64:128, :, 129:130], ones16[64:128, 0:NP, :])

            loop_cm = tc.For_i(0, loop_n, 1) if loop_n else contextlib.nullcontext()
            with loop_cm:
                nc.vector.memset(acc, 0.0)

                # ---------------- PASS A ----------------
                for st_i in range(n_stripes):
                    s0 = st_i * STRIPE
                    xt = xtpool.tile([128, KT, STRIPE], FP16)
                    eqs, vas = [], []
                    # all 4 chunks: load + fp16-convert + transpose up front
                    for c in range(CPS):
                        cs = slice(c * 128, (c + 1) * 128)
                        xc = xpool.tile([128, H], F32)
                        nc.sync.dma_start(
                            out=xc, in_=x_d[s0 + c * 128 : s0 + (c + 1) * 128, :]
                        )
                        xh = xhpool.tile([128, H], FP16)
                        nc.scalar.activation(xh, xc, COPYF)
                        for g in range(KT // 4):
                            pt = tp.tile([128, 4, 128], FP16)
                            for kk in range(4):
                                k = g * 4 + kk
                                nc.tensor.transpose(
                                    pt[:, kk, :],
                                    xh[:, k * 128 : (k + 1) * 128],
                                    identh,
                                )
                            nc.vector.tensor_copy(
                                xt[:, g * 4 : (g + 1) * 4, c * 128 : (c + 1) * 128],
                                pt,
                            )

                    # QV chunk blocks with K-proj tiles interleaved: the K
                    # matmuls cover the Q/V psum drain latency between chunks
                    for c in range(CPS):
                        sc = st_i * CPS + c
                        cs = slice(c * 128, (c + 1) * 128)
                        eqc = eqpool.tile([128, H], FP16, tag="eq")
                        vac = vapool.tile([128, NH, 65], FP16, tag="va")
                        eqs.append(eqc)
                        vas.append(vac)
                        mb = mask_sb[:, sc : sc + 1]
                        # Q/V interleaved k-outer: 4 matmuls share each
                        # stationary xt[:,k,cs] -> 1 weight load per k
                        pqs = [
                            proj.tile([128, 512], F32, tag="proj", name=f"pq{i}")
                            for i in range(4)
                        ]
                        for k in range(KT):
                            for i, (wr_, half) in enumerate(
                                ((wq_r, 0), (wq_r, 1), (wv_r, 0), (wv_r, 1))
                            ):
                                hs = slice(half * 512, (half + 1) * 512)
                                nc.tensor.matmul(
                                    pqs[i],
                                    xt[:, k, cs],
                                    wr_[:, k, hs],
                                    start=k == 0,
                                    stop=k == KT - 1,
                                )
                        for half in range(2):
                            hs = slice(half * 512, (half + 1) * 512)
                            pq = pqs[half]
                            if with_bias:
                                nc.vector.tensor_add(pq, pq, bqb[:, hs])
                            nc.scalar.activation(eqc[:, hs], pq, EXPF, bias=mb)
                        for half in range(2):
                            hs = slice(half * 512, (half + 1) * 512)
                            pv = pqs[2 + half]
                            dst = vac[:, half * 8 : (half + 1) * 8, 0:64]
                            src = pv[:].rearrange("p (h e) -> p h e", e=64)
                            if with_bias:
                                nc.vector.tensor_add(
                                    dst,
                                    src,
                                    bvb[:, hs].rearrange("p (h e) -> p h e", e=64),
                                )
                            else:
                                nc.vector.tensor_copy(dst, src)
                        nc.vector.tensor_copy(vac[:, :, 64:65], ones16)

                        # two K-proj tiles after each QV chunk block
                        for t in (2 * c, 2 * c + 1):
                            pk = ktp.tile([128, STRIPE], F32, tag="pk")
                            for k in range(KT):
                                nc.tensor.matmul(
                                    pk,
                                    wk_r[:, k, t * 128 : (t + 1) * 128],
                                    xt[:, k, :],
                                    start=k == 0,
                                    stop=k == KT - 1,
                                )
                            if with_bias:
                                nc.scalar.activation(
                                    ekt_sb[:, t, s0 : s0 + STRIPE],
                                    pk,
                                    EXPF,
                                    bias=bkc[:, t : t + 1],
                                )
                            else:
                                nc.scalar.activation(
                                    ekt_sb[:, t, s0 : s0 + STRIPE], pk, EXPF
                                )

                    # phase 2: s_ctx accumulation, chained over the stripe
                    for hp in range(NP):
                        p2 = p2p.tile([128, 130], F32, tag="pt")
                        for c in range(CPS):
                            nc.tensor.matmul(
                                p2,
                                eqs[c][:, hp * 128 : (hp + 1) * 128],
                                vas[c][:, hp * 2 : hp * 2 + 2, :],
                                start=c == 0,
                                stop=c == CPS - 1,
                            )
                        nc.vector.tensor_add(acc[:, hp, :], acc[:, hp, :], p2)

                # ---------------- finalize s_ctx -> fp16 block-diag ------
                rr = small.tile([128, NP], F32, tag="rr")
                nc.vector.reciprocal(rr[0:64, :], acc[0:64, :, 64])
                nc.vector.reciprocal(rr[64:128, :], acc[64:128, :, 129])
                nc.vector.tensor_tensor(
                    out=sctx[0:64, :, 0:64],
                    in0=acc[0:64, :, 0:64],
                    in1=_bcast(rr[0:64, :], 64),
                    op=mybir.AluOpType.mult,
                )
                nc.vector.tensor_tensor(
                    out=sctx[64:128, :, 64:128],
                    in0=acc[64:128, :, 65:129],
                    in1=_bcast(rr[64:128, :], 64),
                    op=mybir.AluOpType.mult,
                )

                # ---------------- PASS B ----------------
                for cc in range(n_chunks):
                    cs = slice(cc * 128, (cc + 1) * 128)
                    ob = opool.tile([128, H], F32)
                    for pp in range(NP // 2):
                        pool, ptag = ((proj, "proj"), (ktp, "pk"))[pp % 2]
                        p3 = pool.tile([128, 2, 130], F32, tag=ptag)
                        for j in range(2):
                            nc.tensor.matmul(
                                p3[:, j, :],
                                ekt_sb[:, 2 * pp + j, cs],
                                sctx[:, 2 * pp + j, :],
                                start=True,
                                stop=True,
                            )
                        r4 = small.tile([128, 2, 2], F32, tag="r4")
                        nc.vector.reciprocal(r4, p3[:, :, 128:130])
                        dst = ob[:, pp * 256 : (pp + 1) * 256].rearrange(
                            "p (j h e) -> p j h e", j=2, e=64
                        )
                        rb = bass.AP(
                            tensor=r4.tensor,
                            offset=r4.offset,
                            ap=[r4.ap[0], r4.ap[1], r4.ap[2], [0, 64]],
                        )
                        nc.vector.tensor_tensor(
                            out=dst,
                            in0=p3[:, :, 0:128].rearrange(
                                "p j (h e) -> p j h e", e=64
                            ),
                            in1=rb,
                            op=mybir.AluOpType.mult,
                        )
                    nc.gpsimd.dma_start(
                        out=out_d[cc * 128 : (cc + 1) * 128, :], in_=ob
                    )

    nc.compile()
    return nc


_CACHE = {}


def _get_nc(seq_len, with_bias):
    key = (seq_len, with_bias)
    if key not in _CACHE:
        _CACHE[key] = build_kernel(seq_len, with_bias)
    return _CACHE[key]


def kernel(hidden_states, attention_mask, Wq, bq, Wk, bk, Wv, bv):
    hidden_states = np.asarray(hidden_states, dtype=np.float32)
    attention_mask = np.asarray(attention_mask, dtype=np.float32)
    Wq = np.asarray(Wq, dtype=np.float32)
    Wk = np.asarray(Wk, dtype=np.float32)
    Wv = np.asarray(Wv, dtype=np.float32)
    bq = np.asarray(bq, dtype=np.float32)
    bk = np.asarray(bk, dtype=np.float32)
    bv = np.asarray(bv, dtype=np.float32)
    b, s, h = hidden_states.shape
    with_bias = bool(bq.any() or bk.any() or bv.any())
    nc = _get_nc(s, with_bias)

    mask = attention_mask.reshape(b, s)
    in_maps = []
    for i in range(b):
        m = {
            "x": np.ascontiguousarray(hidden_states[i]),
            "mask": np.ascontiguousarray(mask[i]),
            "wq": Wq,
            "wk": Wk,
            "wv": Wv,
        }
        if with_bias:
            m.update({"bq": bq, "bk": bk, "bv": bv})
        in_maps.append(m)

    res = bass_utils.run_bass_kernel_spmd(nc, in_maps, core_ids=list(range(b)))
    return np.stack([res.results[i]["out"] for i in range(b)], axis=0)
